# revision 48
# baseline (speedup 1.0000x reference)
"""Trainium2 Bass kernel for additive (tanh) attention with mask.

Computation (per batch b):
    wah    = h @ W_ah.T                             [B, H]
    e      = tanh(wah[:, None, :] + p_att_feats)    [B, M, H]
    logits = e @ w_alpha                            [B, M]
    logits = where(mask == 0, -1e9, logits)
    alpha  = softmax(logits, -1)
    att    = alpha @ att_feats                      [B, D]

Strategy: pure data-parallel over batch (8 batches / core on 8 cores).
Masked rows contribute exactly 0 to the softmax-weighted sum, so only
the ~50% of rows with mask==1 are streamed, in bf16 (~21 MB/core).

v3: the mask is host-known, so the HOST compacts the unmasked rows
into a dense [R_total, H+D] bf16 stream (zero padding to 16-row
granularity per slot) and the device gathers it with SWDGE dma_gather
using a STATIC IOTA index.  Measured on this part, dma_gather is the
only DMA primitive that sustains full stream bandwidth (~330 GB/s/core
vs ~80-110 GB/s for plain HWDGE/SWDGE dma_start of the same bytes with
per-partition-contiguous descriptors -- those serialize an HBM
round-trip per large descriptor; the gather's 5KB-row multi-packet
descriptors pipeline).  Iota over compacted rows also lets consecutive
descriptors coalesce, beating the v1 scattered-row gather (~64us vs
~90us DMA-only per pass).

W^T/h^T ship as one packed bf16 [128, 4160] tensor gathered FIRST on
the same queue (the previous HWDGE f32r load would now stall slot-0
compute).  exp() is applied without max-subtraction (logits bounded);
masked/pad rows get an additive -1e9 bias (one [128,K] DVE add + one
batched exp per slot) so their exp underflows to exactly 0.  The
weighted-sum matmuls run bf16 x bf16 -> fp32 PSUM accumulating over
row chunks; normalization by 1/sum is applied once at PSUM drain.  The
per-chunk tanh-energy ops (add, tanh, dot) run in bf16 for DVE 2x
mode.  Cross-partition softmax reduction is a DVE-only log-tree +
32x32 stream transpose (keeps Pool free for gather descriptors).
Phase-1 SBUF pools are opened before the phase-0 scratch pool so the
stack allocator gives them non-overlapping addresses.

Numerics vs fp32 reference: rel-err ~3e-3 (bf16 quantization),
gate is 2e-2.

Self-contained: hardcodes B=64, M=1024, RNN=1024, H=512, D=2048, 8 cores.
"""

import numpy as np

import concourse.bacc as bacc
import concourse.bass as bass
import concourse.mybir as mybir
from concourse import library_config
from concourse.bass_utils import run_bass_kernel_spmd
from concourse.tile import TileContext

B, M, RNN, H, D = 64, 1024, 1024, 512, 2048
NCORES = 8
BL = B // NCORES  # batches per core
CW = H + D  # combined row width (elements)
NEG = -1e9
F32 = mybir.dt.float32
BF16 = mybir.dt.bfloat16
I16 = mybir.dt.int16
RC = RNN // 128  # 8
WTW = RC * H + RC * BL + 64  # packed wt+ht row width, padded so
# WTW * 2 bytes is a multiple of 256 (dma_gather elem_size constraint)


def _plan(mask: np.ndarray):
    """Assign batches to (core, slot) balanced by unmasked count; compute
    per-slot padded row counts (identical across cores - SPMD)."""
    n = mask.sum(axis=1).astype(np.int64)  # [B]
    order = np.argsort(-n, kind="stable")
    batch_of = np.empty((NCORES, BL), dtype=np.int64)
    for j in range(BL):
        for c in range(NCORES):
            batch_of[c, j] = order[j * NCORES + c]
    nbar = np.empty(BL, dtype=np.int64)
    for j in range(BL):
        mx = max(int(n[batch_of[c, j]]) for c in range(NCORES))
        nbar[j] = ((max(mx, 1) + 15) // 16) * 16  # multiple of 16 for idx wrap
    nch = [(int(v) + 127) // 128 for v in nbar]
    return batch_of, n, nbar, nch


def _build(nbar, nch, reps=1, bench_mode=False, loop_n=0, fsplit=2,
           ring=32768, fbufs=3, probe=frozenset()):
    """Build the SPMD bass program (same for all cores).  reps>1 repeats
    phase 1 (benchmark amplification only; outputs are overwritten).
    bench_mode replaces the bulk comb input with device-side zero-filled
    internal DRAM so per-call host transfer is tiny."""
    nbar = [int(v) for v in nbar]
    nch = [int(v) for v in nch]
    stot = int(sum(v // 16 for v in nbar))  # iota idx columns (int16)
    tch = int(sum(nch))  # total chunks (bias columns)
    soff = np.cumsum([0] + [v // 16 for v in nbar])
    boff = np.cumsum([0] + list(nch))
    roff = np.cumsum([0] + nbar)  # compacted row offset per slot
    TOT = int(roff[-1])
    max_nch = max(nch)

    nc = bacc.Bacc(
        "TRN2", target_bir_lowering=False, dynamic_dma_scratch_size=ring
    )
    # host-compacted unmasked rows (p | feats), slot-major, zero-padded
    if bench_mode:
        comb_d = nc.dram_tensor("comb_i", [TOT, CW], BF16)
    else:
        comb_d = nc.dram_tensor("comb", [TOT, CW], BF16, kind="ExternalInput")
    # packed weights: row p = [wt(p, rc, hh) for rc,hh] + [ht(p, rc, b)];
    # wt[p, rc, hh] = W[hh, rc*128+p], ht[p, rc, b] = h[b, rc*128+p]
    wtht_d = nc.dram_tensor("wtht", [128, WTW], BF16, kind="ExternalInput")
    wa_d = nc.dram_tensor("walpha", [1, H], BF16, kind="ExternalInput")
    # oh[b, j*128+p] = (b == j): one-hot lhsT used to broadcast row j of the
    # [BL, H] wah tile to all 128 partitions without any SBUF->SBUF move
    oh_d = nc.dram_tensor("oh", [BL, BL * 128], BF16, kind="ExternalInput")
    # iota gather indices (slot streams) + 8 cols for the wtht gather
    idx_d = nc.dram_tensor("idx", [128, stot + 8], I16, kind="ExternalInput")
    bias_d = nc.dram_tensor("bias", [128, tch], F32, kind="ExternalInput")
    ones_d = nc.dram_tensor("ones", [1, 128], BF16, kind="ExternalInput")
    # all BL outputs ship as ONE DMA at pass end -- per-slot 8KB HWDGE
    # stores cost ~2-5us each on the slow qSPDynamicHW ring and were
    # serializing ~18us/pass.  Output is in transposed tile layout
    # out[p, j*16 + t] = att_j[t*128 + p]; the host untransposes.
    out_d = nc.dram_tensor("out", [128, BL * 16], F32, kind="ExternalOutput")

    with TileContext(nc) as tc:
        nc.gpsimd.load_library(library_config.mlp)
        with (
            tc.tile_pool(name="const", bufs=1) as cp,
            tc.tile_pool(name="fp", bufs=fbufs) as fp,
            tc.tile_pool(name="lp", bufs=4) as lp,
            tc.tile_pool(name="wk", bufs=4) as wk,
            tc.tile_pool(name="sm", bufs=3) as sm,
            tc.tile_pool(name="op", bufs=1) as op,
        ):
            idx_t = cp.tile([128, stot + 8], I16)
            nc.sync.dma_start(idx_t[:, :], idx_d[:, :])
            if bench_mode:
                # zero-fill the internal bulk tensor once (phase -1)
                with tc.tile_pool(name="fill", bufs=1) as fillp:
                    ztf = fillp.tile([128, CW], BF16)
                    nc.vector.memset(ztf[:, :], 0.0)
                    nblk = TOT // 128
                    for blk in range(nblk):
                        nc.sync.dma_start(
                            comb_d[blk * 128 : (blk + 1) * 128, :], ztf[:, :]
                        )
                    if TOT - nblk * 128:
                        nc.sync.dma_start(
                            comb_d[nblk * 128 :, :], ztf[: TOT - nblk * 128, :]
                        )
            bias_t = cp.tile([128, tch], F32)
            nc.sync.dma_start(bias_t[:, :], bias_d[:, :])
            wahb = cp.tile([128, BL, H], BF16)  # per-slot wah broadcast
            walphab = cp.tile([128, H], BF16)  # w_alpha broadcast
            # all-ones f32 [128, 128]: one matmul ones.T @ rowsum replaces
            # the DVE partition-reduce tree AND broadcasts the softmax
            # denominator to every partition
            ones128f = cp.tile([128, 128], F32)
            nc.vector.memset(ones128f[:, :], 1.0)
            # zero bf16 operands for the group-opening matmul (see below)
            z128 = cp.tile([1, 128], BF16)
            nc.vector.memset(z128[:, :], 0.0)
            z16 = cp.tile([1, 16], BF16)
            nc.vector.memset(z16[:, :], 0.0)
            if "no_dma" in probe:
                fconst = cp.tile([128, max_nch, CW], BF16)
                nc.vector.memset(fconst[:, :, :], 0.0)
            # pre-zero the gather buffers once: pad rows use negative idx
            # entries (gather skips them entirely), so the pad regions keep
            # whatever the buffer holds -- which must be finite bf16
            for _ in range(fbufs):
                fz = fp.tile([128, max_nch, CW], BF16, tag="f")
                nc.vector.memset(fz[:, :, :], 0.0)
            if "no_exp" in probe:
                exconst = cp.tile([128, max_nch], BF16)
                nc.vector.memset(exconst[:, :], 0.001)

            # ---------------- phase 0: wah = h @ W.T, broadcasts ----------
            with (
                tc.tile_pool(name="ph0", bufs=1) as p0,
                tc.tile_pool(name="ph0ps", bufs=2, space="PSUM") as p0ps,
            ):
                # packed weights arrive via the fast gather path (iota over
                # 128 rows) so slot-0 compute is not stalled by slow HWDGE
                wtht_t = p0.tile([128, 1, WTW], BF16)
                nc.gpsimd.dma_gather(
                    wtht_t[:, :, :], wtht_d[:, :],
                    idx_t[:, stot : stot + 8],
                    128, 128, WTW, single_packet=False,
                )
                ones_sb = p0.tile([1, 128], BF16)
                nc.sync.dma_start(ones_sb[:, :], ones_d[:, :])
                oh_sb = p0.tile([BL, BL * 128], BF16)
                nc.sync.dma_start(oh_sb[:, :], oh_d[:, :])
                wa_sb = p0.tile([1, H], BF16)
                nc.sync.dma_start(wa_sb[:, :], wa_d[:, :])

                # wah [b, h] = sum_rc ht.T @ wt
                ps_wah = p0ps.tile([BL, H], F32, tag="wah")
                for rc in range(RC):
                    nc.tensor.matmul(
                        ps_wah[:, :],
                        wtht_t[:, 0, RC * H + rc * BL : RC * H + (rc + 1) * BL],
                        wtht_t[:, 0, rc * H : (rc + 1) * H],
                        start=(rc == 0),
                        stop=(rc == RC - 1),
                    )
                wah_sb = p0.tile([BL, H], BF16)
                nc.vector.tensor_copy(wah_sb[:, :], ps_wah[:, :])
                # broadcast row j to 128 partitions: onehot_j.T @ wah_sb
                for j in range(BL):
                    pb = p0ps.tile([128, H], F32, tag="bc")
                    nc.tensor.matmul(
                        pb[:, :],
                        oh_sb[:, j * 128 : (j + 1) * 128],
                        wah_sb[:, :],
                        start=True, stop=True,
                    )
                    nc.scalar.copy(wahb[:, j, :], pb[:, :])
                pb = p0ps.tile([128, H], F32, tag="bc")
                nc.tensor.matmul(
                    pb[:, :], ones_sb[:, :], wa_sb[:, :], start=True, stop=True
                )
                nc.scalar.copy(walphab[:, :], pb[:, :])

            # ---------------- phase 1: per-slot sparse attention ----------
            def issue_f_gather(j):
                nj, cj = nbar[j], nch[j]
                f_t = fp.tile([128, max_nch, CW], BF16, tag="f")
                # split the gather so the pipeline starts on the first
                # piece while the rest streams
                s0 = int(soff[j])
                per = max(1, (cj + fsplit - 1) // fsplit)
                c0 = 0
                while c0 < cj:
                    c1 = min(cj, c0 + per)
                    r0, r1 = c0 * 128, min(nj, c1 * 128)
                    nc.gpsimd.dma_gather(
                        f_t[:, c0:c1, :], comb_d[:, :],
                        idx_t[:, s0 + r0 // 16 : s0 + r1 // 16],
                        r1 - r0, r1 - r0, CW, single_packet=False,
                    )
                    c0 = c1
                return f_t

            import contextlib

            with tc.tile_pool(
                name="aps", bufs=(1 if "old_mm" in probe else 2), space="PSUM"
            ) as aps:
                loop_cm = (
                    tc.For_i(0, loop_n, 1,
                             hint_engines=tuple(mybir.ALL_ENGINES))
                    if loop_n else contextlib.nullcontext()
                )
                with loop_cm:
                  for rep in range(reps):
                    att_all = op.tile([128, BL * 16], F32, tag="at")
                    if "no_dma" not in probe:
                        pending_f = issue_f_gather(0)
                    for j in range(BL):
                        nj, cj = nbar[j], nch[j]
                        if "no_dma" in probe:
                            f_t = fconst
                        else:
                            f_t = pending_f
                            if j + 1 < BL:
                                pending_f = issue_f_gather(j + 1)
                        if "no_cmp" in probe:
                            continue

                        logits = lp.tile([128, max_nch], F32, tag="lg")
                        nc.vector.memset(logits[:, :], 0.0)
                        # transposed weighted-sum accumulator: ps2[p, t]
                        # = att[t*128 + p].  Spans all 128 partitions so
                        # the PSUM drain is 16 elements wide, not 2048.
                        # PSUM tiles are allocated full-bank ([128, 512]
                        # f32 = one 2KB bank) -- matmul start=True clears
                        # has_written at BANK granularity, so distinct
                        # accumulation groups must never share a bank.
                        # Within ps2's bank, the 16 column-groups are
                        # opened by ONE [128,16] zero-matmul with
                        # start=True; all real matmuls accumulate.
                        if "old_mm" in probe:
                            ps2 = aps.tile([1, D], F32, tag="att")
                        else:
                            ps2 = aps.tile([128, 512], F32, tag="att")
                            nc.tensor.matmul(
                                ps2[:, 0:16], z128[:, :], z16[:, :],
                                start=True, stop=False,
                                skip_group_check=True,
                            )
                        # ops are emitted batched BY TYPE (all adds, all
                        # tanhs, all dots): engine queues are in-order, so
                        # interleaving a-t-s per chunk head-of-line-blocks
                        # each engine on the previous chunk's cross-engine
                        # dep; batching lets every engine stream.
                        evec = []
                        for c in range(cj):
                            kc = min(128, nj - c * 128)
                            e = wk.tile([128, H], BF16, tag=f"e{c}")
                            evec.append(e)
                            nc.vector.tensor_add(
                                e[:kc, :], f_t[:kc, c, 0:H], wahb[:kc, j, :]
                            )
                        if "no_tanh" not in probe:
                            for c in range(cj):
                                kc = min(128, nj - c * 128)
                                nc.scalar.activation(
                                    evec[c][:kc, :], evec[c][:kc, :],
                                    mybir.ActivationFunctionType.Tanh,
                                )
                        # fused mult+mult with row-sum accumulation:
                        # logits[:, c] = sum_h e*walpha  (NOTE:
                        # InstTensorTensorReduce crashes the NRT exec on
                        # this runtime; scalar_tensor_tensor is fine.)
                        if "no_stt" not in probe:
                            for c in range(cj):
                                kc = min(128, nj - c * 128)
                                tt = wk.tile([128, H], BF16, tag="tt")
                                nc.vector.scalar_tensor_tensor(
                                    out=tt[:kc, :],
                                    in0=evec[c][:kc, :],
                                    scalar=1.0,
                                    in1=walphab[:kc, :],
                                    op0=mybir.AluOpType.mult,
                                    op1=mybir.AluOpType.mult,
                                    accum_out=logits[:kc, c : c + 1],
                                )
                        # exp(logits + bias) per chunk so the weighted
                        # matmuls of chunk c can stream on the PE while
                        # later chunks are still in tanh/stt; bias = -1e9
                        # on masked/pad rows so their exp == 0.  The bf16
                        # output doubles as the PE weight dtype; accum_out
                        # collects per-chunk rowsums for the denominator.
                        if "no_exp" in probe:
                            exr = exconst
                        else:
                            exr = lp.tile([128, max_nch], BF16, tag="exr")
                            rsall = lp.tile([128, max_nch], F32, tag="rsa")
                            for c in range(cj):
                                nc.scalar.activation(
                                    exr[:, c : c + 1],
                                    logits[:, c : c + 1],
                                    mybir.ActivationFunctionType.Exp,
                                    bias=bias_t[
                                        :, int(boff[j]) + c : int(boff[j]) + c + 1
                                    ],
                                    accum_out=rsall[:, c : c + 1],
                                )
                        # weighted sum, transposed: the f_t 128-col block is
                        # the STATIONARY operand and the exp column is the
                        # moving one, so ps2[p, t] = sum_k f[k, t*128+p] *
                        # exp[k] accumulates across all partitions.
                        if "no_mm" not in probe:
                            for c in range(cj):
                                kc = min(128, nj - c * 128)
                                if "old_mm" in probe:
                                    for d in range(D // 512):
                                        nc.tensor.matmul(
                                            ps2[0:1, d * 512 : (d + 1) * 512],
                                            exr[:kc, c : c + 1],
                                            f_t[:kc, c, H + d * 512 : H + (d + 1) * 512],
                                            start=(c == 0),
                                            stop=(c == cj - 1),
                                        )
                                    continue
                                for t in range(D // 128):
                                    nc.tensor.matmul(
                                        ps2[:, t : t + 1],
                                        f_t[:kc, c, H + t * 128 : H + (t + 1) * 128],
                                        exr[:kc, c : c + 1],
                                        start=False,
                                        stop=(c == cj - 1),
                                        skip_group_check=True,
                                    )
                        # softmax denominator: reduce the per-chunk rowsums,
                        # then ones.T @ rowsum sums across partitions AND
                        # broadcasts the result to every partition in one
                        # matmul; DVE reciprocal gives the per-partition 1/s
                        # for the drain scale.
                        if "no_tree" not in probe and "no_exp" not in probe:
                            rowsum = sm.tile([128, 1], F32, tag="rs")
                            nc.vector.tensor_reduce(
                                rowsum[:, :], rsall[:, :cj],
                                axis=mybir.AxisListType.X,
                                op=mybir.AluOpType.add,
                            )
                            ps_r = aps.tile([128, 512], F32, tag="sr")
                            nc.tensor.matmul(
                                ps_r[:, 0:1], ones128f[:, :], rowsum[:, :],
                                start=True, stop=True,
                            )
                            rinvb = sm.tile([128, 1], F32, tag="ri")
                            nc.vector.reciprocal(rinvb[:, :], ps_r[:, 0:1])
                        if "no_drain" not in probe and "no_mm" not in probe:
                            no_ri = "no_tree" in probe or "no_exp" in probe
                            if "old_mm" in probe:
                                attw = op.tile([1, D], F32, tag="aw")
                                nc.scalar.activation(
                                    attw[:, :], ps2[0:1, :],
                                    mybir.ActivationFunctionType.Copy,
                                    scale=(1.0 if no_ri else rinvb[0:1, 0:1]),
                                )
                                for t in range(D // 128):
                                    nc.vector.tensor_copy(
                                        att_all[0:1, j * 16 + t : j * 16 + t + 1],
                                        attw[0:1, t * 128 : t * 128 + 1],
                                    )
                            else:
                                nc.scalar.activation(
                                    att_all[:, j * 16 : (j + 1) * 16],
                                    ps2[:, 0:16],
                                    mybir.ActivationFunctionType.Copy,
                                    scale=(1.0 if no_ri else rinvb[:, 0:1]),
                                )
                    if "no_drain" not in probe and "no_mm" not in probe \
                            and "no_cmp" not in probe:
                        nc.sync.dma_start(out_d[:, :], att_all[:, :])
    nc.compile()
    return nc


_CACHE: dict = {}


def _get_compiled(mask: np.ndarray):
    import os

    probe = frozenset(os.environ.get("KERNEL_PROBE", "").split(",")) - {""}
    key = (mask.tobytes(), probe)
    hit = _CACHE.get("key") == key
    if not hit:
        batch_of, n, nbar, nch = _plan(mask)
        nc = _build(nbar, nch, probe=probe)
        _CACHE.update(
            key=key, nc=nc, batch_of=batch_of, n=n, nbar=nbar, nch=nch
        )
    return _CACHE


def kernel(h, att_feats, att_mask, p_att_feats, W_ah, w_alpha):
    h = np.ascontiguousarray(np.asarray(h, dtype=np.float32))
    att_feats = np.asarray(att_feats, dtype=np.float32)
    mask = np.asarray(att_mask).astype(np.int32)
    p_att_feats = np.asarray(p_att_feats, dtype=np.float32)
    W_ah = np.ascontiguousarray(np.asarray(W_ah, dtype=np.float32))
    w_alpha = np.ascontiguousarray(np.asarray(w_alpha, dtype=np.float32))

    st = _get_compiled(mask)
    nc, batch_of, n, nbar, nch = (
        st["nc"], st["batch_of"], st["n"], st["nbar"], st["nch"]
    )
    stot = int(sum(int(v) // 16 for v in nbar))
    tch = int(sum(nch))
    soff = np.cumsum([0] + [int(v) // 16 for v in nbar])
    boff = np.cumsum([0] + list(nch))
    roff = np.cumsum([0] + [int(v) for v in nbar])
    TOT = int(roff[-1])

    import ml_dtypes

    bf16 = ml_dtypes.bfloat16
    ones = np.ones((1, 128), dtype=bf16)
    oh = np.zeros((BL, BL * 128), dtype=bf16)
    for j in range(BL):
        oh[j, j * 128 : (j + 1) * 128] = 1.0
    wa_row = w_alpha.reshape(1, H).astype(bf16)
    wt_arr = (
        W_ah.T.reshape(RC, 128, H).transpose(1, 0, 2).reshape(128, RC * H)
    )

    wblk = np.arange(128, dtype=np.int64).reshape(8, 16).T.astype(np.int16)

    in_maps = []
    for c in range(NCORES):
        bids = batch_of[c]
        bias_arr = np.full((128, tch), NEG, dtype=np.float32)
        comb = np.zeros((TOT, CW), dtype=bf16)
        # iota gather indices; pad rows (i >= nb) get -1 so the gather
        # skips their transfers entirely (trailing negatives are ignored)
        idx_arr = np.zeros((128, stot + 8), dtype=np.int16)
        for j in range(BL):
            b = int(bids[j])
            nb = int(n[b])
            nj = int(nbar[j])
            pad = np.arange(nj, dtype=np.int64) + int(roff[j])
            pad[nb:] = -1
            blk = pad.reshape(nj // 16, 16).T.astype(np.int16)  # [16, nj/16]
            idx_arr[:, int(soff[j]) : int(soff[j + 1])] = np.tile(blk, (8, 1))
        idx_arr[:, stot : stot + 8] = np.tile(wblk, (8, 1))
        for j in range(BL):
            b = int(bids[j])
            nb = int(n[b])
            rows = np.nonzero(mask[b])[0]
            r0 = int(roff[j])
            comb[r0 : r0 + nb, :H] = p_att_feats[b][rows]
            comb[r0 : r0 + nb, H:] = att_feats[b][rows]
            # bias: 0 for real rows (c*128 + p < nb), -1e9 otherwise
            valid = (
                np.arange(128)[:, None] + 128 * np.arange(nch[j])[None, :] < nb
            )
            bias_arr[:, int(boff[j]) : int(boff[j]) + nch[j]][valid] = 0.0
        h_l = h[bids]  # [BL, RNN]
        ht_arr = (
            h_l.T.reshape(RC, 128, BL).transpose(1, 0, 2).reshape(128, RC * BL)
        )
        wtht = np.concatenate(
            [wt_arr, ht_arr, np.zeros((128, 64), dtype=np.float32)], axis=1
        ).astype(bf16)
        in_maps.append(
            {
                "comb": comb,
                "wtht": wtht,
                "walpha": wa_row,
                "idx": idx_arr,
                "bias": bias_arr,
                "ones": ones,
                "oh": oh,
            }
        )

    res = run_bass_kernel_spmd(nc, in_maps, core_ids=list(range(NCORES)))
    kernel._last_results = res  # for test harness introspection

    out = np.empty((B, D), dtype=np.float32)
    for c in range(NCORES):
        o = res.results[c]["out"]  # [128, BL*16], o[p, j*16+t] = att[t*128+p]
        for j in range(BL):
            out[int(batch_of[c, j])] = (
                o[:, j * 16 : (j + 1) * 16].T.reshape(D)
            )
    return out


# revision 50
# speedup vs baseline: 1.0372x; 1.0372x over previous
"""Trainium2 Bass kernel for additive (tanh) attention with mask.

Computation (per batch b):
    wah    = h @ W_ah.T                             [B, H]
    e      = tanh(wah[:, None, :] + p_att_feats)    [B, M, H]
    logits = e @ w_alpha                            [B, M]
    logits = where(mask == 0, -1e9, logits)
    alpha  = softmax(logits, -1)
    att    = alpha @ att_feats                      [B, D]

Strategy: pure data-parallel over batch (8 batches / core on 8 cores).
Masked rows contribute exactly 0 to the softmax-weighted sum, so only
the ~50% of rows with mask==1 are streamed, in bf16 (~21 MB/core).

v3: the mask is host-known, so the HOST compacts the unmasked rows
into a dense [R_total, H+D] bf16 stream (16-row slot granularity,
pad rows carry idx=-1 so the gather skips their transfers) and the
device gathers it with SWDGE dma_gather using a STATIC IOTA index.
Measured on this part, dma_gather is the only DMA primitive that
sustains full stream bandwidth (~330 GB/s/core vs ~80-110 GB/s for
plain HWDGE/SWDGE dma_start of the same bytes with
per-partition-contiguous descriptors -- those serialize an HBM
round-trip per large descriptor; the gather's 5KB-row multi-packet
descriptors pipeline).  Iota over compacted rows beats the v1
scattered-row gather (~65us vs ~90us DMA-only per pass).

W^T/h^T ship as one packed bf16 [128, 4224] tensor gathered FIRST on
the same queue (an HWDGE f32r load would stall slot-0 compute).
exp() is applied without max-subtraction (logits bounded); masked/pad
rows get an additive -1e9 bias via the per-chunk exp's bias operand so
their exp underflows to exactly 0; its accum_out yields the rowsums
for free.  Energy ops run in bf16 (DVE 2x) and are emitted batched BY
TYPE (all adds, all tanhs, all dots per slot) -- engine queues are
in-order, so interleaving per chunk head-of-line-blocks each engine on
the previous chunk's cross-engine dep.

The weighted sum runs TRANSPOSED: the f_t 128-column block is the
stationary operand and the exp column the moving one, accumulating
ps2[p, t] = att[t*128+p] across all partitions, so the PSUM drain is a
[128, 16] activation (the [1, 2048] single-partition drain plus
per-slot 8KB HWDGE stores cost ~18us/pass).  PSUM gotcha: matmul
start=True clears has_written at BANK granularity, so the 16
interleaved column-groups are opened by ONE [128,16] zero-matmul and
all real matmuls accumulate (start=False); PSUM tiles are full-bank so
pool buffers never share a bank.  The softmax denominator is a single
ones.T @ rowsum matmul (sums across partitions AND broadcasts 1/s
everywhere, replacing a DVE log-tree); all BL outputs leave as ONE
pass-end DMA in transposed layout (host untransposes).  Phase-1 SBUF
pools are opened before the phase-0 scratch pool so the stack
allocator gives them non-overlapping addresses.

Measured (paired For_i slope, 8 cores concurrent): ~69-72us/core vs
96.6us baseline; DMA-only floor ~65us (~320 GB/s/core, HBM-per-NC
limit is ~358).  Numerics vs fp32 reference: rel-err ~2.9e-3 (bf16
quantization), gate is 2e-2.

Self-contained: hardcodes B=64, M=1024, RNN=1024, H=512, D=2048, 8 cores.
"""

import numpy as np

import concourse.bacc as bacc
import concourse.bass as bass
import concourse.mybir as mybir
from concourse import library_config
from concourse.bass_utils import run_bass_kernel_spmd
from concourse.tile import TileContext

B, M, RNN, H, D = 64, 1024, 1024, 512, 2048
NCORES = 8
BL = B // NCORES  # batches per core
CW = H + D  # combined row width (elements)
NEG = -1e9
F32 = mybir.dt.float32
BF16 = mybir.dt.bfloat16
I16 = mybir.dt.int16
RC = RNN // 128  # 8
WTW = RC * H + RC * BL + 64  # packed wt+ht row width, padded so
# WTW * 2 bytes is a multiple of 256 (dma_gather elem_size constraint)


def _plan(mask: np.ndarray):
    """Assign batches to (core, slot) balanced by unmasked count; compute
    per-slot padded row counts (identical across cores - SPMD)."""
    n = mask.sum(axis=1).astype(np.int64)  # [B]
    order = np.argsort(-n, kind="stable")
    batch_of = np.empty((NCORES, BL), dtype=np.int64)
    for j in range(BL):
        for c in range(NCORES):
            batch_of[c, j] = order[j * NCORES + c]
    nbar = np.empty(BL, dtype=np.int64)
    for j in range(BL):
        mx = max(int(n[batch_of[c, j]]) for c in range(NCORES))
        nbar[j] = ((max(mx, 1) + 15) // 16) * 16  # multiple of 16 for idx wrap
    nch = [(int(v) + 127) // 128 for v in nbar]
    return batch_of, n, nbar, nch


def _build(nbar, nch, reps=1, bench_mode=False, loop_n=0, fsplit=2,
           ring=32768, fbufs=3, probe=frozenset()):
    """Build the SPMD bass program (same for all cores).  reps>1 repeats
    phase 1 (benchmark amplification only; outputs are overwritten).
    bench_mode replaces the bulk comb input with device-side zero-filled
    internal DRAM so per-call host transfer is tiny."""
    nbar = [int(v) for v in nbar]
    nch = [int(v) for v in nch]
    stot = int(sum(v // 16 for v in nbar))  # iota idx columns (int16)
    tch = int(sum(nch))  # total chunks (bias columns)
    soff = np.cumsum([0] + [v // 16 for v in nbar])
    boff = np.cumsum([0] + list(nch))
    roff = np.cumsum([0] + nbar)  # compacted row offset per slot
    TOT = int(roff[-1])
    max_nch = max(nch)

    nc = bacc.Bacc(
        "TRN2", target_bir_lowering=False, dynamic_dma_scratch_size=ring
    )
    # host-compacted unmasked rows (p | feats), slot-major, zero-padded
    if bench_mode:
        comb_d = nc.dram_tensor("comb_i", [TOT, CW], BF16)
    else:
        comb_d = nc.dram_tensor("comb", [TOT, CW], BF16, kind="ExternalInput")
    # packed weights: row p = [wt(p, rc, hh) for rc,hh] + [ht(p, rc, b)];
    # wt[p, rc, hh] = W[hh, rc*128+p], ht[p, rc, b] = h[b, rc*128+p]
    wtht_d = nc.dram_tensor("wtht", [128, WTW], BF16, kind="ExternalInput")
    wa_d = nc.dram_tensor("walpha", [1, H], BF16, kind="ExternalInput")
    # oh[b, j*128+p] = (b == j): one-hot lhsT used to broadcast row j of the
    # [BL, H] wah tile to all 128 partitions without any SBUF->SBUF move
    oh_d = nc.dram_tensor("oh", [BL, BL * 128], BF16, kind="ExternalInput")
    # iota gather indices (slot streams) + 8 cols for the wtht gather
    idx_d = nc.dram_tensor("idx", [128, stot + 8], I16, kind="ExternalInput")
    bias_d = nc.dram_tensor("bias", [128, tch], F32, kind="ExternalInput")
    ones_d = nc.dram_tensor("ones", [1, 128], BF16, kind="ExternalInput")
    # all BL outputs ship as ONE DMA at pass end -- per-slot 8KB HWDGE
    # stores cost ~2-5us each on the slow qSPDynamicHW ring and were
    # serializing ~18us/pass.  Output is in transposed tile layout
    # out[p, j*16 + t] = att_j[t*128 + p]; the host untransposes.
    out_d = nc.dram_tensor("out", [128, BL * 16], F32, kind="ExternalOutput")

    with TileContext(nc) as tc:
        nc.gpsimd.load_library(library_config.mlp)
        with (
            tc.tile_pool(name="const", bufs=1) as cp,
            tc.tile_pool(name="fp", bufs=fbufs) as fp,
            tc.tile_pool(name="lp", bufs=4) as lp,
            tc.tile_pool(name="wk", bufs=4) as wk,
            tc.tile_pool(name="sm", bufs=3) as sm,
            tc.tile_pool(name="op", bufs=1) as op,
        ):
            idx_t = cp.tile([128, stot + 8], I16)
            nc.sync.dma_start(idx_t[:, :], idx_d[:, :])
            if bench_mode:
                # zero-fill the internal bulk tensor once (phase -1)
                with tc.tile_pool(name="fill", bufs=1) as fillp:
                    ztf = fillp.tile([128, CW], BF16)
                    nc.vector.memset(ztf[:, :], 0.0)
                    nblk = TOT // 128
                    for blk in range(nblk):
                        nc.sync.dma_start(
                            comb_d[blk * 128 : (blk + 1) * 128, :], ztf[:, :]
                        )
                    if TOT - nblk * 128:
                        nc.sync.dma_start(
                            comb_d[nblk * 128 :, :], ztf[: TOT - nblk * 128, :]
                        )
            bias_t = cp.tile([128, tch], F32)
            nc.sync.dma_start(bias_t[:, :], bias_d[:, :])
            wahb = cp.tile([128, BL, H], BF16)  # per-slot wah broadcast
            walphab = cp.tile([128, H], BF16)  # w_alpha broadcast
            # all-ones f32 [128, 128]: one matmul ones.T @ rowsum replaces
            # the DVE partition-reduce tree AND broadcasts the softmax
            # denominator to every partition
            ones128f = cp.tile([128, 128], F32)
            nc.vector.memset(ones128f[:, :], 1.0)
            # zero bf16 operands for the group-opening matmul (see below)
            z128 = cp.tile([1, 128], BF16)
            nc.vector.memset(z128[:, :], 0.0)
            z16 = cp.tile([1, 16], BF16)
            nc.vector.memset(z16[:, :], 0.0)
            if "no_dma" in probe:
                fconst = cp.tile([128, max_nch, CW], BF16)
                nc.vector.memset(fconst[:, :, :], 0.0)
            # pre-zero the gather buffers once: pad rows use negative idx
            # entries (gather skips them entirely), so the pad regions keep
            # whatever the buffer holds -- which must be finite bf16
            for _ in range(fbufs):
                fz = fp.tile([128, max_nch, CW], BF16, tag="f")
                nc.vector.memset(fz[:, :, :], 0.0)
            if "no_exp" in probe:
                exconst = cp.tile([128, max_nch], BF16)
                nc.vector.memset(exconst[:, :], 0.001)

            # ---------------- phase 0: wah = h @ W.T, broadcasts ----------
            with (
                tc.tile_pool(name="ph0", bufs=1) as p0,
                tc.tile_pool(name="ph0ps", bufs=2, space="PSUM") as p0ps,
            ):
                # packed weights arrive via the fast gather path (iota over
                # 128 rows) so slot-0 compute is not stalled by slow HWDGE
                wtht_t = p0.tile([128, 1, WTW], BF16)
                nc.gpsimd.dma_gather(
                    wtht_t[:, :, :], wtht_d[:, :],
                    idx_t[:, stot : stot + 8],
                    128, 128, WTW, single_packet=False,
                )
                ones_sb = p0.tile([1, 128], BF16)
                nc.sync.dma_start(ones_sb[:, :], ones_d[:, :])
                oh_sb = p0.tile([BL, BL * 128], BF16)
                nc.sync.dma_start(oh_sb[:, :], oh_d[:, :])
                wa_sb = p0.tile([1, H], BF16)
                nc.sync.dma_start(wa_sb[:, :], wa_d[:, :])

                # wah [b, h] = sum_rc ht.T @ wt
                ps_wah = p0ps.tile([BL, H], F32, tag="wah")
                for rc in range(RC):
                    nc.tensor.matmul(
                        ps_wah[:, :],
                        wtht_t[:, 0, RC * H + rc * BL : RC * H + (rc + 1) * BL],
                        wtht_t[:, 0, rc * H : (rc + 1) * H],
                        start=(rc == 0),
                        stop=(rc == RC - 1),
                    )
                wah_sb = p0.tile([BL, H], BF16)
                nc.vector.tensor_copy(wah_sb[:, :], ps_wah[:, :])
                # broadcast row j to 128 partitions: onehot_j.T @ wah_sb
                for j in range(BL):
                    pb = p0ps.tile([128, H], F32, tag="bc")
                    nc.tensor.matmul(
                        pb[:, :],
                        oh_sb[:, j * 128 : (j + 1) * 128],
                        wah_sb[:, :],
                        start=True, stop=True,
                    )
                    nc.scalar.copy(wahb[:, j, :], pb[:, :])
                pb = p0ps.tile([128, H], F32, tag="bc")
                nc.tensor.matmul(
                    pb[:, :], ones_sb[:, :], wa_sb[:, :], start=True, stop=True
                )
                nc.scalar.copy(walphab[:, :], pb[:, :])

            # ---------------- phase 1: per-slot sparse attention ----------
            def issue_f_gather(j):
                nj, cj = nbar[j], nch[j]
                f_t = fp.tile([128, max_nch, CW], BF16, tag="f")
                # split the gather so the pipeline starts on the first
                # piece while the rest streams
                s0 = int(soff[j])
                per = max(1, (cj + fsplit - 1) // fsplit)
                c0 = 0
                while c0 < cj:
                    c1 = min(cj, c0 + per)
                    r0, r1 = c0 * 128, min(nj, c1 * 128)
                    nc.gpsimd.dma_gather(
                        f_t[:, c0:c1, :], comb_d[:, :],
                        idx_t[:, s0 + r0 // 16 : s0 + r1 // 16],
                        r1 - r0, r1 - r0, CW, single_packet=False,
                    )
                    c0 = c1
                return f_t

            import contextlib

            with tc.tile_pool(
                name="aps", bufs=(1 if "old_mm" in probe else 2), space="PSUM"
            ) as aps:
                loop_cm = (
                    tc.For_i(0, loop_n, 1,
                             hint_engines=tuple(mybir.ALL_ENGINES))
                    if loop_n else contextlib.nullcontext()
                )
                with loop_cm:
                  for rep in range(reps):
                    att_all = op.tile([128, BL * 16], F32, tag="at")
                    if "no_dma" not in probe:
                        pending_f = issue_f_gather(0)
                    for j in range(BL):
                        nj, cj = nbar[j], nch[j]
                        if "no_dma" in probe:
                            f_t = fconst
                        else:
                            f_t = pending_f
                            if j + 1 < BL:
                                pending_f = issue_f_gather(j + 1)
                        if "no_cmp" in probe:
                            continue

                        logits = lp.tile([128, max_nch], F32, tag="lg")
                        nc.vector.memset(logits[:, :], 0.0)
                        # transposed weighted-sum accumulator: ps2[p, t]
                        # = att[t*128 + p].  Spans all 128 partitions so
                        # the PSUM drain is 16 elements wide, not 2048.
                        # PSUM tiles are allocated full-bank ([128, 512]
                        # f32 = one 2KB bank) -- matmul start=True clears
                        # has_written at BANK granularity, so distinct
                        # accumulation groups must never share a bank.
                        # Within ps2's bank, the 16 column-groups are
                        # opened by ONE [128,16] zero-matmul with
                        # start=True; all real matmuls accumulate.
                        if "old_mm" in probe:
                            ps2 = aps.tile([1, D], F32, tag="att")
                        else:
                            ps2 = aps.tile([128, 512], F32, tag="att")
                            nc.tensor.matmul(
                                ps2[:, 0:16], z128[:, :], z16[:, :],
                                start=True, stop=False,
                                skip_group_check=True,
                            )
                        # ops are emitted batched BY TYPE (all adds, all
                        # tanhs, all dots): engine queues are in-order, so
                        # interleaving a-t-s per chunk head-of-line-blocks
                        # each engine on the previous chunk's cross-engine
                        # dep; batching lets every engine stream.
                        evec = []
                        for c in range(cj):
                            kc = min(128, nj - c * 128)
                            e = wk.tile([128, H], BF16, tag=f"e{c}")
                            evec.append(e)
                            if "no_add" not in probe:
                                nc.vector.tensor_add(
                                    e[:kc, :], f_t[:kc, c, 0:H], wahb[:kc, j, :]
                                )
                        if "no_tanh" not in probe:
                            for c in range(cj):
                                kc = min(128, nj - c * 128)
                                if "no_add" in probe:
                                    nc.scalar.activation(
                                        evec[c][:kc, :], f_t[:kc, c, 0:H],
                                        mybir.ActivationFunctionType.Tanh,
                                    )
                                else:
                                    nc.scalar.activation(
                                        evec[c][:kc, :], evec[c][:kc, :],
                                        mybir.ActivationFunctionType.Tanh,
                                    )
                        # fused mult+mult with row-sum accumulation:
                        # logits[:, c] = sum_h e*walpha  (NOTE:
                        # InstTensorTensorReduce crashes the NRT exec on
                        # this runtime; scalar_tensor_tensor is fine.)
                        if "no_stt" not in probe:
                            for c in range(cj):
                                kc = min(128, nj - c * 128)
                                tt = wk.tile([128, H], BF16, tag="tt")
                                nc.vector.scalar_tensor_tensor(
                                    out=tt[:kc, :],
                                    in0=evec[c][:kc, :],
                                    scalar=1.0,
                                    in1=walphab[:kc, :],
                                    op0=mybir.AluOpType.mult,
                                    op1=mybir.AluOpType.mult,
                                    accum_out=logits[:kc, c : c + 1],
                                )
                        # exp(logits + bias) per chunk so the weighted
                        # matmuls of chunk c can stream on the PE while
                        # later chunks are still in tanh/stt; bias = -1e9
                        # on masked/pad rows so their exp == 0.  The bf16
                        # output doubles as the PE weight dtype; accum_out
                        # collects per-chunk rowsums for the denominator.
                        if "no_exp" in probe:
                            exr = exconst
                        else:
                            exr = lp.tile([128, max_nch], BF16, tag="exr")
                            rsall = lp.tile([128, max_nch], F32, tag="rsa")
                            for c in range(cj):
                                nc.scalar.activation(
                                    exr[:, c : c + 1],
                                    logits[:, c : c + 1],
                                    mybir.ActivationFunctionType.Exp,
                                    bias=bias_t[
                                        :, int(boff[j]) + c : int(boff[j]) + c + 1
                                    ],
                                    accum_out=rsall[:, c : c + 1],
                                )
                        # weighted sum, transposed: the f_t 128-col block is
                        # the STATIONARY operand and the exp column is the
                        # moving one, so ps2[p, t] = sum_k f[k, t*128+p] *
                        # exp[k] accumulates across all partitions.
                        if "no_mm" not in probe:
                            for c in range(cj):
                                kc = min(128, nj - c * 128)
                                if "old_mm" in probe:
                                    for d in range(D // 512):
                                        nc.tensor.matmul(
                                            ps2[0:1, d * 512 : (d + 1) * 512],
                                            exr[:kc, c : c + 1],
                                            f_t[:kc, c, H + d * 512 : H + (d + 1) * 512],
                                            start=(c == 0),
                                            stop=(c == cj - 1),
                                        )
                                    continue
                                for t in range(D // 128):
                                    nc.tensor.matmul(
                                        ps2[:, t : t + 1],
                                        f_t[:kc, c, H + t * 128 : H + (t + 1) * 128],
                                        exr[:kc, c : c + 1],
                                        start=False,
                                        stop=(c == cj - 1),
                                        skip_group_check=True,
                                    )
                        # softmax denominator: reduce the per-chunk rowsums,
                        # then ones.T @ rowsum sums across partitions AND
                        # broadcasts the result to every partition in one
                        # matmul; DVE reciprocal gives the per-partition 1/s
                        # for the drain scale.
                        if "no_tree" not in probe and "no_exp" not in probe:
                            rowsum = sm.tile([128, 1], F32, tag="rs")
                            nc.vector.tensor_reduce(
                                rowsum[:, :], rsall[:, :cj],
                                axis=mybir.AxisListType.X,
                                op=mybir.AluOpType.add,
                            )
                            ps_r = aps.tile([128, 512], F32, tag="sr")
                            nc.tensor.matmul(
                                ps_r[:, 0:1], ones128f[:, :], rowsum[:, :],
                                start=True, stop=True,
                            )
                            rinvb = sm.tile([128, 1], F32, tag="ri")
                            nc.vector.reciprocal(rinvb[:, :], ps_r[:, 0:1])
                        if "no_drain" not in probe and "no_mm" not in probe:
                            no_ri = "no_tree" in probe or "no_exp" in probe
                            if "old_mm" in probe:
                                attw = op.tile([1, D], F32, tag="aw")
                                nc.scalar.activation(
                                    attw[:, :], ps2[0:1, :],
                                    mybir.ActivationFunctionType.Copy,
                                    scale=(1.0 if no_ri else rinvb[0:1, 0:1]),
                                )
                                for t in range(D // 128):
                                    nc.vector.tensor_copy(
                                        att_all[0:1, j * 16 + t : j * 16 + t + 1],
                                        attw[0:1, t * 128 : t * 128 + 1],
                                    )
                            else:
                                nc.scalar.activation(
                                    att_all[:, j * 16 : (j + 1) * 16],
                                    ps2[:, 0:16],
                                    mybir.ActivationFunctionType.Copy,
                                    scale=(1.0 if no_ri else rinvb[:, 0:1]),
                                )
                    if "no_drain" not in probe and "no_mm" not in probe \
                            and "no_cmp" not in probe:
                        nc.sync.dma_start(out_d[:, :], att_all[:, :])
    nc.compile()
    return nc


_CACHE: dict = {}


def _get_compiled(mask: np.ndarray):
    import os

    probe = frozenset(os.environ.get("KERNEL_PROBE", "").split(",")) - {""}
    key = (mask.tobytes(), probe)
    hit = _CACHE.get("key") == key
    if not hit:
        batch_of, n, nbar, nch = _plan(mask)
        nc = _build(nbar, nch, probe=probe)
        _CACHE.update(
            key=key, nc=nc, batch_of=batch_of, n=n, nbar=nbar, nch=nch
        )
    return _CACHE


def kernel(h, att_feats, att_mask, p_att_feats, W_ah, w_alpha):
    h = np.ascontiguousarray(np.asarray(h, dtype=np.float32))
    att_feats = np.asarray(att_feats, dtype=np.float32)
    mask = np.asarray(att_mask).astype(np.int32)
    p_att_feats = np.asarray(p_att_feats, dtype=np.float32)
    W_ah = np.ascontiguousarray(np.asarray(W_ah, dtype=np.float32))
    w_alpha = np.ascontiguousarray(np.asarray(w_alpha, dtype=np.float32))

    st = _get_compiled(mask)
    nc, batch_of, n, nbar, nch = (
        st["nc"], st["batch_of"], st["n"], st["nbar"], st["nch"]
    )
    stot = int(sum(int(v) // 16 for v in nbar))
    tch = int(sum(nch))
    soff = np.cumsum([0] + [int(v) // 16 for v in nbar])
    boff = np.cumsum([0] + list(nch))
    roff = np.cumsum([0] + [int(v) for v in nbar])
    TOT = int(roff[-1])

    import ml_dtypes

    bf16 = ml_dtypes.bfloat16
    ones = np.ones((1, 128), dtype=bf16)
    oh = np.zeros((BL, BL * 128), dtype=bf16)
    for j in range(BL):
        oh[j, j * 128 : (j + 1) * 128] = 1.0
    wa_row = w_alpha.reshape(1, H).astype(bf16)
    wt_arr = (
        W_ah.T.reshape(RC, 128, H).transpose(1, 0, 2).reshape(128, RC * H)
    )

    wblk = np.arange(128, dtype=np.int64).reshape(8, 16).T.astype(np.int16)

    in_maps = []
    for c in range(NCORES):
        bids = batch_of[c]
        bias_arr = np.full((128, tch), NEG, dtype=np.float32)
        comb = np.zeros((TOT, CW), dtype=bf16)
        # iota gather indices; pad rows (i >= nb) get -1 so the gather
        # skips their transfers entirely (trailing negatives are ignored)
        idx_arr = np.zeros((128, stot + 8), dtype=np.int16)
        for j in range(BL):
            b = int(bids[j])
            nb = int(n[b])
            nj = int(nbar[j])
            pad = np.arange(nj, dtype=np.int64) + int(roff[j])
            pad[nb:] = -1
            blk = pad.reshape(nj // 16, 16).T.astype(np.int16)  # [16, nj/16]
            idx_arr[:, int(soff[j]) : int(soff[j + 1])] = np.tile(blk, (8, 1))
        idx_arr[:, stot : stot + 8] = np.tile(wblk, (8, 1))
        for j in range(BL):
            b = int(bids[j])
            nb = int(n[b])
            rows = np.nonzero(mask[b])[0]
            r0 = int(roff[j])
            comb[r0 : r0 + nb, :H] = p_att_feats[b][rows]
            comb[r0 : r0 + nb, H:] = att_feats[b][rows]
            # bias: 0 for real rows (c*128 + p < nb), -1e9 otherwise
            valid = (
                np.arange(128)[:, None] + 128 * np.arange(nch[j])[None, :] < nb
            )
            bias_arr[:, int(boff[j]) : int(boff[j]) + nch[j]][valid] = 0.0
        h_l = h[bids]  # [BL, RNN]
        ht_arr = (
            h_l.T.reshape(RC, 128, BL).transpose(1, 0, 2).reshape(128, RC * BL)
        )
        wtht = np.concatenate(
            [wt_arr, ht_arr, np.zeros((128, 64), dtype=np.float32)], axis=1
        ).astype(bf16)
        in_maps.append(
            {
                "comb": comb,
                "wtht": wtht,
                "walpha": wa_row,
                "idx": idx_arr,
                "bias": bias_arr,
                "ones": ones,
                "oh": oh,
            }
        )

    res = run_bass_kernel_spmd(nc, in_maps, core_ids=list(range(NCORES)))
    kernel._last_results = res  # for test harness introspection

    out = np.empty((B, D), dtype=np.float32)
    for c in range(NCORES):
        o = res.results[c]["out"]  # [128, BL*16], o[p, j*16+t] = att[t*128+p]
        for j in range(BL):
            out[int(batch_of[c, j])] = (
                o[:, j * 16 : (j + 1) * 16].T.reshape(D)
            )
    return out


# revision 51
# speedup vs baseline: 1.0744x; 1.0359x over previous
"""Trainium2 Bass kernel for additive (tanh) attention with mask.

Computation (per batch b):
    wah    = h @ W_ah.T                             [B, H]
    e      = tanh(wah[:, None, :] + p_att_feats)    [B, M, H]
    logits = e @ w_alpha                            [B, M]
    logits = where(mask == 0, -1e9, logits)
    alpha  = softmax(logits, -1)
    att    = alpha @ att_feats                      [B, D]

Strategy: pure data-parallel over batch (8 batches / core on 8 cores).
Masked rows contribute exactly 0 to the softmax-weighted sum, so only
the ~50% of rows with mask==1 are streamed, in bf16 (~21 MB/core).

v3: the mask is host-known, so the HOST compacts the unmasked rows
into a dense [R_total, H+D] bf16 stream (16-row slot granularity,
pad rows carry idx=-1 so the gather skips their transfers) and the
device gathers it with SWDGE dma_gather using a STATIC IOTA index.
Measured on this part, dma_gather is the only DMA primitive that
sustains full stream bandwidth (~330 GB/s/core vs ~80-110 GB/s for
plain HWDGE/SWDGE dma_start of the same bytes with
per-partition-contiguous descriptors -- those serialize an HBM
round-trip per large descriptor; the gather's 5KB-row multi-packet
descriptors pipeline).  Iota over compacted rows beats the v1
scattered-row gather (~65us vs ~90us DMA-only per pass).

W^T/h^T ship as one packed bf16 [128, 4224] tensor gathered FIRST on
the same queue (an HWDGE f32r load would stall slot-0 compute).
exp() is applied without max-subtraction (logits bounded); masked/pad
rows get an additive -1e9 bias via the per-chunk exp's bias operand so
their exp underflows to exactly 0; its accum_out yields the rowsums
for free.  Energy ops run in bf16 (DVE 2x) and are emitted batched BY
TYPE (all adds, all tanhs, all dots per slot) -- engine queues are
in-order, so interleaving per chunk head-of-line-blocks each engine on
the previous chunk's cross-engine dep.

The weighted sum runs TRANSPOSED: the f_t 128-column block is the
stationary operand and the exp column the moving one, accumulating
ps2[p, t] = att[t*128+p] across all partitions, so the PSUM drain is a
[128, 16] activation (the [1, 2048] single-partition drain plus
per-slot 8KB HWDGE stores cost ~18us/pass).  PSUM gotcha: matmul
start=True clears has_written at BANK granularity, so the 16
interleaved column-groups are opened by ONE [128,16] zero-matmul and
all real matmuls accumulate (start=False); PSUM tiles are full-bank so
pool buffers never share a bank.  The softmax denominator is a single
ones.T @ rowsum matmul (sums across partitions AND broadcasts 1/s
everywhere, replacing a DVE log-tree); all BL outputs leave as ONE
pass-end DMA in transposed layout (host untransposes).  Phase-1 SBUF
pools are opened before the phase-0 scratch pool so the stack
allocator gives them non-overlapping addresses.

Measured (paired For_i slope, 8 cores concurrent): ~69-72us/core vs
96.6us baseline; DMA-only floor ~65us (~320 GB/s/core, HBM-per-NC
limit is ~358).  Numerics vs fp32 reference: rel-err ~2.9e-3 (bf16
quantization), gate is 2e-2.

Self-contained: hardcodes B=64, M=1024, RNN=1024, H=512, D=2048, 8 cores.
"""

import numpy as np

import concourse.bacc as bacc
import concourse.bass as bass
import concourse.mybir as mybir
from concourse import library_config
from concourse.bass_utils import run_bass_kernel_spmd
from concourse.tile import TileContext

B, M, RNN, H, D = 64, 1024, 1024, 512, 2048
NCORES = 8
BL = B // NCORES  # batches per core
CW = H + D  # combined row width (elements)
NEG = -1e9
F32 = mybir.dt.float32
BF16 = mybir.dt.bfloat16
I16 = mybir.dt.int16
RC = RNN // 128  # 8
WTW = RC * H + RC * BL + 64  # packed wt+ht row width, padded so
# WTW * 2 bytes is a multiple of 256 (dma_gather elem_size constraint)


def _plan(mask: np.ndarray):
    """Assign batches to (core, slot) balanced by unmasked count; compute
    per-slot padded row counts (identical across cores - SPMD)."""
    n = mask.sum(axis=1).astype(np.int64)  # [B]
    order = np.argsort(-n, kind="stable")
    batch_of = np.empty((NCORES, BL), dtype=np.int64)
    for j in range(BL):
        for c in range(NCORES):
            batch_of[c, j] = order[j * NCORES + c]
    nbar = np.empty(BL, dtype=np.int64)
    for j in range(BL):
        mx = max(int(n[batch_of[c, j]]) for c in range(NCORES))
        nbar[j] = ((max(mx, 1) + 15) // 16) * 16  # multiple of 16 for idx wrap
    nch = [(int(v) + 127) // 128 for v in nbar]
    return batch_of, n, nbar, nch


def _build(nbar, nch, reps=1, bench_mode=False, loop_n=0, fsplit=2,
           ring=32768, fbufs=3, probe=frozenset()):
    """Build the SPMD bass program (same for all cores).  reps>1 repeats
    phase 1 (benchmark amplification only; outputs are overwritten).
    bench_mode replaces the bulk comb input with device-side zero-filled
    internal DRAM so per-call host transfer is tiny."""
    nbar = [int(v) for v in nbar]
    nch = [int(v) for v in nch]
    stot = int(sum(v // 16 for v in nbar))  # iota idx columns (int16)
    tch = int(sum(nch))  # total chunks (bias columns)
    soff = np.cumsum([0] + [v // 16 for v in nbar])
    boff = np.cumsum([0] + list(nch))
    roff = np.cumsum([0] + nbar)  # compacted row offset per slot
    TOT = int(roff[-1])
    max_nch = max(nch)

    nc = bacc.Bacc(
        "TRN2", target_bir_lowering=False, dynamic_dma_scratch_size=ring
    )
    # host-compacted unmasked rows (p | feats), slot-major, zero-padded
    if bench_mode:
        comb_d = nc.dram_tensor("comb_i", [TOT, CW], BF16)
    else:
        comb_d = nc.dram_tensor("comb", [TOT, CW], BF16, kind="ExternalInput")
    # packed weights: row p = [wt(p, rc, hh) for rc,hh] + [ht(p, rc, b)];
    # wt[p, rc, hh] = W[hh, rc*128+p], ht[p, rc, b] = h[b, rc*128+p]
    wtht_d = nc.dram_tensor("wtht", [128, WTW], BF16, kind="ExternalInput")
    wa_d = nc.dram_tensor("walpha", [1, H], BF16, kind="ExternalInput")
    # oh[b, j*128+p] = (b == j): one-hot lhsT used to broadcast row j of the
    # [BL, H] wah tile to all 128 partitions without any SBUF->SBUF move
    oh_d = nc.dram_tensor("oh", [BL, BL * 128], BF16, kind="ExternalInput")
    # iota gather indices (slot streams) + 8 cols for the wtht gather
    idx_d = nc.dram_tensor("idx", [128, stot + 8], I16, kind="ExternalInput")
    bias_d = nc.dram_tensor("bias", [128, tch], F32, kind="ExternalInput")
    ones_d = nc.dram_tensor("ones", [1, 128], BF16, kind="ExternalInput")
    # all BL outputs ship as ONE DMA at pass end -- per-slot 8KB HWDGE
    # stores cost ~2-5us each on the slow qSPDynamicHW ring and were
    # serializing ~18us/pass.  Output is in transposed tile layout
    # out[p, j*16 + t] = att_j[t*128 + p]; the host untransposes.
    out_d = nc.dram_tensor("out", [128, BL * 16], F32, kind="ExternalOutput")

    with TileContext(nc) as tc:
        nc.gpsimd.load_library(library_config.mlp)
        with (
            tc.tile_pool(name="const", bufs=1) as cp,
            tc.tile_pool(name="fp", bufs=fbufs) as fp,
            tc.tile_pool(name="lp", bufs=4) as lp,
            tc.tile_pool(name="wk", bufs=4) as wk,
            tc.tile_pool(name="sm", bufs=3) as sm,
            tc.tile_pool(name="op", bufs=2) as op,
        ):
            idx_t = cp.tile([128, stot + 8], I16)
            nc.sync.dma_start(idx_t[:, :], idx_d[:, :])
            if bench_mode:
                # zero-fill the internal bulk tensor once (phase -1)
                with tc.tile_pool(name="fill", bufs=1) as fillp:
                    ztf = fillp.tile([128, CW], BF16)
                    nc.vector.memset(ztf[:, :], 0.0)
                    nblk = TOT // 128
                    for blk in range(nblk):
                        nc.sync.dma_start(
                            comb_d[blk * 128 : (blk + 1) * 128, :], ztf[:, :]
                        )
                    if TOT - nblk * 128:
                        nc.sync.dma_start(
                            comb_d[nblk * 128 :, :], ztf[: TOT - nblk * 128, :]
                        )
            bias_t = cp.tile([128, tch], F32)
            nc.sync.dma_start(bias_t[:, :], bias_d[:, :])
            wahb = cp.tile([128, BL, H], BF16)  # per-slot wah broadcast
            walphab = cp.tile([128, H], BF16)  # w_alpha broadcast
            # all-ones f32 [128, 128]: one matmul ones.T @ rowsum replaces
            # the DVE partition-reduce tree AND broadcasts the softmax
            # denominator to every partition
            ones128f = cp.tile([128, 128], F32)
            nc.vector.memset(ones128f[:, :], 1.0)
            # zero bf16 operands for the group-opening matmul (see below)
            z128 = cp.tile([1, 128], BF16)
            nc.vector.memset(z128[:, :], 0.0)
            z16 = cp.tile([1, 16], BF16)
            nc.vector.memset(z16[:, :], 0.0)
            if "no_dma" in probe:
                fconst = cp.tile([128, max_nch, CW], BF16)
                nc.vector.memset(fconst[:, :, :], 0.0)
            # pre-zero the gather buffers once: pad rows use negative idx
            # entries (gather skips them entirely), so the pad regions keep
            # whatever the buffer holds -- which must be finite bf16
            for _ in range(fbufs):
                fz = fp.tile([128, max_nch, CW], BF16, tag="f")
                nc.vector.memset(fz[:, :, :], 0.0)
            if "no_exp" in probe:
                exconst = cp.tile([128, max_nch], BF16)
                nc.vector.memset(exconst[:, :], 0.001)

            # ---------------- phase 0: wah = h @ W.T, broadcasts ----------
            with (
                tc.tile_pool(name="ph0", bufs=1) as p0,
                tc.tile_pool(name="ph0ps", bufs=2, space="PSUM") as p0ps,
            ):
                # packed weights arrive via the fast gather path (iota over
                # 128 rows) so slot-0 compute is not stalled by slow HWDGE
                wtht_t = p0.tile([128, 1, WTW], BF16)
                nc.gpsimd.dma_gather(
                    wtht_t[:, :, :], wtht_d[:, :],
                    idx_t[:, stot : stot + 8],
                    128, 128, WTW, single_packet=False,
                )
                ones_sb = p0.tile([1, 128], BF16)
                nc.sync.dma_start(ones_sb[:, :], ones_d[:, :])
                oh_sb = p0.tile([BL, BL * 128], BF16)
                nc.sync.dma_start(oh_sb[:, :], oh_d[:, :])
                wa_sb = p0.tile([1, H], BF16)
                nc.sync.dma_start(wa_sb[:, :], wa_d[:, :])

                # wah [b, h] = sum_rc ht.T @ wt
                ps_wah = p0ps.tile([BL, H], F32, tag="wah")
                for rc in range(RC):
                    nc.tensor.matmul(
                        ps_wah[:, :],
                        wtht_t[:, 0, RC * H + rc * BL : RC * H + (rc + 1) * BL],
                        wtht_t[:, 0, rc * H : (rc + 1) * H],
                        start=(rc == 0),
                        stop=(rc == RC - 1),
                    )
                wah_sb = p0.tile([BL, H], BF16)
                nc.vector.tensor_copy(wah_sb[:, :], ps_wah[:, :])
                # broadcast row j to 128 partitions: onehot_j.T @ wah_sb
                for j in range(BL):
                    pb = p0ps.tile([128, H], F32, tag="bc")
                    nc.tensor.matmul(
                        pb[:, :],
                        oh_sb[:, j * 128 : (j + 1) * 128],
                        wah_sb[:, :],
                        start=True, stop=True,
                    )
                    nc.scalar.copy(wahb[:, j, :], pb[:, :])
                pb = p0ps.tile([128, H], F32, tag="bc")
                nc.tensor.matmul(
                    pb[:, :], ones_sb[:, :], wa_sb[:, :], start=True, stop=True
                )
                nc.scalar.copy(walphab[:, :], pb[:, :])

            # ---------------- phase 1: per-slot sparse attention ----------
            def issue_f_gather(j):
                nj, cj = nbar[j], nch[j]
                f_t = fp.tile([128, max_nch, CW], BF16, tag="f")
                # split the gather so the pipeline starts on the first
                # piece while the rest streams
                s0 = int(soff[j])
                per = max(1, (cj + fsplit - 1) // fsplit)
                c0 = 0
                while c0 < cj:
                    c1 = min(cj, c0 + per)
                    r0, r1 = c0 * 128, min(nj, c1 * 128)
                    nc.gpsimd.dma_gather(
                        f_t[:, c0:c1, :], comb_d[:, :],
                        idx_t[:, s0 + r0 // 16 : s0 + r1 // 16],
                        r1 - r0, r1 - r0, CW, single_packet=False,
                    )
                    c0 = c1
                return f_t

            import contextlib

            with tc.tile_pool(
                name="aps", bufs=(1 if "old_mm" in probe else 2), space="PSUM"
            ) as aps:
                loop_cm = (
                    tc.For_i(0, loop_n, 1,
                             hint_engines=tuple(mybir.ALL_ENGINES))
                    if loop_n else contextlib.nullcontext()
                )
                with loop_cm:
                  for rep in range(reps):
                    att_all = op.tile([128, BL * 16], F32, tag="at")
                    if "no_dma" not in probe:
                        pending_f = issue_f_gather(0)
                    for j in range(BL):
                        nj, cj = nbar[j], nch[j]
                        if "no_dma" in probe:
                            f_t = fconst
                        else:
                            f_t = pending_f
                            if j + 1 < BL:
                                pending_f = issue_f_gather(j + 1)
                        if "no_cmp" in probe:
                            continue

                        logits = lp.tile([128, max_nch], F32, tag="lg")
                        nc.vector.memset(logits[:, :], 0.0)
                        # transposed weighted-sum accumulator: ps2[p, t]
                        # = att[t*128 + p].  Spans all 128 partitions so
                        # the PSUM drain is 16 elements wide, not 2048.
                        # PSUM tiles are allocated full-bank ([128, 512]
                        # f32 = one 2KB bank) -- matmul start=True clears
                        # has_written at BANK granularity, so distinct
                        # accumulation groups must never share a bank.
                        # Within ps2's bank, the 16 column-groups are
                        # opened by ONE [128,16] zero-matmul with
                        # start=True; all real matmuls accumulate.
                        if "old_mm" in probe:
                            ps2 = aps.tile([1, D], F32, tag="att")
                        else:
                            ps2 = aps.tile([128, 512], F32, tag="att")
                            nc.tensor.matmul(
                                ps2[:, 0:16], z128[:, :], z16[:, :],
                                start=True, stop=False,
                                skip_group_check=True,
                            )
                        # ops are emitted batched BY TYPE (all adds, all
                        # tanhs, all dots): engine queues are in-order, so
                        # interleaving a-t-s per chunk head-of-line-blocks
                        # each engine on the previous chunk's cross-engine
                        # dep; batching lets every engine stream.
                        evec = []
                        for c in range(cj):
                            kc = min(128, nj - c * 128)
                            e = wk.tile([128, H], BF16, tag=f"e{c}")
                            evec.append(e)
                            if "no_add" not in probe:
                                nc.vector.tensor_add(
                                    e[:kc, :], f_t[:kc, c, 0:H], wahb[:kc, j, :]
                                )
                        if "no_tanh" not in probe:
                            for c in range(cj):
                                kc = min(128, nj - c * 128)
                                if "no_add" in probe:
                                    nc.scalar.activation(
                                        evec[c][:kc, :], f_t[:kc, c, 0:H],
                                        mybir.ActivationFunctionType.Tanh,
                                    )
                                else:
                                    nc.scalar.activation(
                                        evec[c][:kc, :], evec[c][:kc, :],
                                        mybir.ActivationFunctionType.Tanh,
                                    )
                        # fused mult+mult with row-sum accumulation:
                        # logits[:, c] = sum_h e*walpha  (NOTE:
                        # InstTensorTensorReduce crashes the NRT exec on
                        # this runtime; scalar_tensor_tensor is fine.)
                        if "no_stt" not in probe:
                            for c in range(cj):
                                kc = min(128, nj - c * 128)
                                tt = wk.tile([128, H], BF16, tag="tt")
                                nc.vector.scalar_tensor_tensor(
                                    out=tt[:kc, :],
                                    in0=evec[c][:kc, :],
                                    scalar=1.0,
                                    in1=walphab[:kc, :],
                                    op0=mybir.AluOpType.mult,
                                    op1=mybir.AluOpType.mult,
                                    accum_out=logits[:kc, c : c + 1],
                                )
                        # exp(logits + bias) per chunk so the weighted
                        # matmuls of chunk c can stream on the PE while
                        # later chunks are still in tanh/stt; bias = -1e9
                        # on masked/pad rows so their exp == 0.  The bf16
                        # output doubles as the PE weight dtype; accum_out
                        # collects per-chunk rowsums for the denominator.
                        if "no_exp" in probe:
                            exr = exconst
                        else:
                            exr = lp.tile([128, max_nch], BF16, tag="exr")
                            rsall = lp.tile([128, max_nch], F32, tag="rsa")
                            for c in range(cj):
                                nc.scalar.activation(
                                    exr[:, c : c + 1],
                                    logits[:, c : c + 1],
                                    mybir.ActivationFunctionType.Exp,
                                    bias=bias_t[
                                        :, int(boff[j]) + c : int(boff[j]) + c + 1
                                    ],
                                    accum_out=rsall[:, c : c + 1],
                                )
                        # weighted sum, transposed: the f_t 128-col block is
                        # the STATIONARY operand and the exp column is the
                        # moving one, so ps2[p, t] = sum_k f[k, t*128+p] *
                        # exp[k] accumulates across all partitions.
                        if "no_mm" not in probe:
                            for c in range(cj):
                                kc = min(128, nj - c * 128)
                                if "old_mm" in probe:
                                    for d in range(D // 512):
                                        nc.tensor.matmul(
                                            ps2[0:1, d * 512 : (d + 1) * 512],
                                            exr[:kc, c : c + 1],
                                            f_t[:kc, c, H + d * 512 : H + (d + 1) * 512],
                                            start=(c == 0),
                                            stop=(c == cj - 1),
                                        )
                                    continue
                                for t in range(D // 128):
                                    nc.tensor.matmul(
                                        ps2[:, t : t + 1],
                                        f_t[:kc, c, H + t * 128 : H + (t + 1) * 128],
                                        exr[:kc, c : c + 1],
                                        start=False,
                                        stop=(c == cj - 1),
                                        skip_group_check=True,
                                    )
                        # softmax denominator: reduce the per-chunk rowsums,
                        # then ones.T @ rowsum sums across partitions AND
                        # broadcasts the result to every partition in one
                        # matmul; DVE reciprocal gives the per-partition 1/s
                        # for the drain scale.
                        if "no_tree" not in probe and "no_exp" not in probe:
                            rowsum = sm.tile([128, 1], F32, tag="rs")
                            nc.vector.tensor_reduce(
                                rowsum[:, :], rsall[:, :cj],
                                axis=mybir.AxisListType.X,
                                op=mybir.AluOpType.add,
                            )
                            ps_r = aps.tile([128, 512], F32, tag="sr")
                            nc.tensor.matmul(
                                ps_r[:, 0:1], ones128f[:, :], rowsum[:, :],
                                start=True, stop=True,
                            )
                            rinvb = sm.tile([128, 1], F32, tag="ri")
                            nc.vector.reciprocal(rinvb[:, :], ps_r[:, 0:1])
                        if "no_drain" not in probe and "no_mm" not in probe:
                            no_ri = "no_tree" in probe or "no_exp" in probe
                            if "old_mm" in probe:
                                attw = op.tile([1, D], F32, tag="aw")
                                nc.scalar.activation(
                                    attw[:, :], ps2[0:1, :],
                                    mybir.ActivationFunctionType.Copy,
                                    scale=(1.0 if no_ri else rinvb[0:1, 0:1]),
                                )
                                for t in range(D // 128):
                                    nc.vector.tensor_copy(
                                        att_all[0:1, j * 16 + t : j * 16 + t + 1],
                                        attw[0:1, t * 128 : t * 128 + 1],
                                    )
                            else:
                                nc.scalar.activation(
                                    att_all[:, j * 16 : (j + 1) * 16],
                                    ps2[:, 0:16],
                                    mybir.ActivationFunctionType.Copy,
                                    scale=(1.0 if no_ri else rinvb[:, 0:1]),
                                )
                    if "no_drain" not in probe and "no_mm" not in probe \
                            and "no_cmp" not in probe:
                        nc.sync.dma_start(out_d[:, :], att_all[:, :])
    nc.compile()
    return nc


_CACHE: dict = {}


def _get_compiled(mask: np.ndarray):
    import os

    probe = frozenset(os.environ.get("KERNEL_PROBE", "").split(",")) - {""}
    key = (mask.tobytes(), probe)
    hit = _CACHE.get("key") == key
    if not hit:
        batch_of, n, nbar, nch = _plan(mask)
        nc = _build(nbar, nch, probe=probe)
        _CACHE.update(
            key=key, nc=nc, batch_of=batch_of, n=n, nbar=nbar, nch=nch
        )
    return _CACHE


def kernel(h, att_feats, att_mask, p_att_feats, W_ah, w_alpha):
    h = np.ascontiguousarray(np.asarray(h, dtype=np.float32))
    att_feats = np.asarray(att_feats, dtype=np.float32)
    mask = np.asarray(att_mask).astype(np.int32)
    p_att_feats = np.asarray(p_att_feats, dtype=np.float32)
    W_ah = np.ascontiguousarray(np.asarray(W_ah, dtype=np.float32))
    w_alpha = np.ascontiguousarray(np.asarray(w_alpha, dtype=np.float32))

    st = _get_compiled(mask)
    nc, batch_of, n, nbar, nch = (
        st["nc"], st["batch_of"], st["n"], st["nbar"], st["nch"]
    )
    stot = int(sum(int(v) // 16 for v in nbar))
    tch = int(sum(nch))
    soff = np.cumsum([0] + [int(v) // 16 for v in nbar])
    boff = np.cumsum([0] + list(nch))
    roff = np.cumsum([0] + [int(v) for v in nbar])
    TOT = int(roff[-1])

    import ml_dtypes

    bf16 = ml_dtypes.bfloat16
    ones = np.ones((1, 128), dtype=bf16)
    oh = np.zeros((BL, BL * 128), dtype=bf16)
    for j in range(BL):
        oh[j, j * 128 : (j + 1) * 128] = 1.0
    wa_row = w_alpha.reshape(1, H).astype(bf16)
    wt_arr = (
        W_ah.T.reshape(RC, 128, H).transpose(1, 0, 2).reshape(128, RC * H)
    )

    wblk = np.arange(128, dtype=np.int64).reshape(8, 16).T.astype(np.int16)

    in_maps = []
    for c in range(NCORES):
        bids = batch_of[c]
        bias_arr = np.full((128, tch), NEG, dtype=np.float32)
        comb = np.zeros((TOT, CW), dtype=bf16)
        # iota gather indices; pad rows (i >= nb) get -1 so the gather
        # skips their transfers entirely (trailing negatives are ignored)
        idx_arr = np.zeros((128, stot + 8), dtype=np.int16)
        for j in range(BL):
            b = int(bids[j])
            nb = int(n[b])
            nj = int(nbar[j])
            pad = np.arange(nj, dtype=np.int64) + int(roff[j])
            pad[nb:] = -1
            blk = pad.reshape(nj // 16, 16).T.astype(np.int16)  # [16, nj/16]
            idx_arr[:, int(soff[j]) : int(soff[j + 1])] = np.tile(blk, (8, 1))
        idx_arr[:, stot : stot + 8] = np.tile(wblk, (8, 1))
        for j in range(BL):
            b = int(bids[j])
            nb = int(n[b])
            rows = np.nonzero(mask[b])[0]
            r0 = int(roff[j])
            comb[r0 : r0 + nb, :H] = p_att_feats[b][rows]
            comb[r0 : r0 + nb, H:] = att_feats[b][rows]
            # bias: 0 for real rows (c*128 + p < nb), -1e9 otherwise
            valid = (
                np.arange(128)[:, None] + 128 * np.arange(nch[j])[None, :] < nb
            )
            bias_arr[:, int(boff[j]) : int(boff[j]) + nch[j]][valid] = 0.0
        h_l = h[bids]  # [BL, RNN]
        ht_arr = (
            h_l.T.reshape(RC, 128, BL).transpose(1, 0, 2).reshape(128, RC * BL)
        )
        wtht = np.concatenate(
            [wt_arr, ht_arr, np.zeros((128, 64), dtype=np.float32)], axis=1
        ).astype(bf16)
        in_maps.append(
            {
                "comb": comb,
                "wtht": wtht,
                "walpha": wa_row,
                "idx": idx_arr,
                "bias": bias_arr,
                "ones": ones,
                "oh": oh,
            }
        )

    res = run_bass_kernel_spmd(nc, in_maps, core_ids=list(range(NCORES)))
    kernel._last_results = res  # for test harness introspection

    out = np.empty((B, D), dtype=np.float32)
    for c in range(NCORES):
        o = res.results[c]["out"]  # [128, BL*16], o[p, j*16+t] = att[t*128+p]
        for j in range(BL):
            out[int(batch_of[c, j])] = (
                o[:, j * 16 : (j + 1) * 16].T.reshape(D)
            )
    return out
